# revision 5
# baseline (speedup 1.0000x reference)
"""Trainium2 Bass kernel for nn_Classifier_custom_12936441496172.

Reference math (per batch b, with av = column-l2-normalized img_b [Cf, R]):
    A      = softmax_r( (vv @ W1) @ av )          # [I, R] attention over R
    F_p    = A @ av.T                             # [I, Cf]
    out[b] = rowsum( (vv @ W2) * F_p )            # [I]

Key identity: out[b, i] = sum_r A[i, r] * ((vv @ W2) @ av)[i, r], so F_p is
never materialized. (vv@W1)@av and (vv@W2)@av come from one stacked weight
matrix qpt, and the column normalization of av folds into a per-column scale
rn[r] = 1/||img_b[:, r]|| applied to the matmul outputs.

Sharding: data-parallel over batch across 8 NeuronCores (16 batches each),
parameters replicated. Parameter prep (vv @ W1/W2, < 1% of FLOPs) on host.

Device kernel per core: 8 groups of 2 batches (N = 512 matmul free dim).
  - img arrives as one [128, 4096] bf16 tile per group (two dma_starts with
    4 KB per-partition lines, issued with a rolling 3-group lookahead so
    drain-critical DMAs never queue behind them).
  - norms: squares at [128, 1024-2048] grain split across ACT/DVE/GpSimd,
    a pairwise bf16 add tree, then the 128-partition reduction: an
    accumulating ones-vector matmul + row Ln/Exp + partition_broadcast for
    the first two (latency-critical) groups, gpsimd partition_all_reduce
    (output already broadcast) + full-tile Ln/Exp for the rest.
    rn = Exp(-0.5 * Ln(n2)) is produced in bf16.
  - main: 5 m-chunks of the 624 stacked rows (tail packs Q/P 56-row
    remainders), each 8 accumulating bf16 matmuls.
  - drains per chunk-pair: sqs = qa*rn (DVE, reads PSUM), E = Exp(sqs) in
    bf16 with fp32 free-axis accum -> sumexp columns (ACT), F = E*rn (DVE
    bf16 2x), then scalar_tensor_tensor F*pa (pa straight from PSUM) with
    accum -> unnormalized output column.
  - softmax denominators are applied per PAIR of groups (reciprocal +
    multiply on [*, 4] tiles + output DMA), so the kernel tail is only the
    last group's drain chain.
All ACT functions (Square/Ln/Exp) are pinned to the one act-func set that
contains all three, so ACT performs a single table load. PE warm-up: ~14
dummy matmuls on a gpsimd-memset tile while the first DMAs are in flight.
Logits are ~N(0,1) so softmax max-subtraction is skipped; exp cannot
overflow fp32.
"""

import numpy as np

_PROGRAM = None

# Problem geometry (hardcoded per contract; kernel.py must be self-contained)
N_CORES = 8
NB = 16          # batches per core
R = 256          # H * W
CF = 1024        # feature channels
KC = CF // 128   # 8 contraction chunks
I = 312          # attributes
G = NB // 2      # groups of 2 batches
N = 2 * R        # matmul moving free dim (2 batches)
TQ = I - 256     # 56-row tails
XW = KC * N      # x tile width (4096)
# m-chunk column offsets in the host-reordered qpt
MCH_Q = [0, 128]       # Q rows 0:128, 128:256
MCH_P = [256, 384]     # P rows 0:128, 128:256
MCH_T = 512            # Q rows 256:312 at cols 512:568, P rows at 568:624
N_WARM = 14
N_MM_NORM = 2    # groups whose norm reduction rides the TensorEngine


def _pin_act_tables(arch):
    """Blank every act-func set except natural_log_exp_and_others (which
    contains Square, Ln and Exp) in the cached table dict, so the table-load
    pass assigns all our activations to that one set and the ACT engine
    performs a single table load instead of flipping per Ln/Exp pair.
    Indices (insertion order) are preserved, so the emitted set id still
    refers to the same canonical act_info.json entry."""
    from concourse.hw_specs import get_activation_tables

    tabs = get_activation_tables(arch)
    target = "natural_log_exp_and_others"
    if target in tabs:
        for name in tabs:
            if name != target:
                tabs[name] = set()


def _build_program():
    import concourse.tile as tile
    from concourse import bacc, bass_isa, mybir

    F32 = mybir.dt.float32
    BF16 = mybir.dt.bfloat16
    MULT = mybir.AluOpType.mult
    EXP = mybir.ActivationFunctionType.Exp
    LN = mybir.ActivationFunctionType.Ln

    nc = bacc.Bacc(
        "TRN2",
        target_bir_lowering=False,
        debug=False,
        enable_asserts=False,
        num_devices=N_CORES,
    )
    _pin_act_tables(nc.m.arch)

    img = nc.dram_tensor("img", [G, 128, XW], BF16, kind="ExternalInput").ap()
    qpt = nc.dram_tensor("qpt", [CF, 2 * I], BF16, kind="ExternalInput").ap()
    out = nc.dram_tensor("out", [I, NB], F32, kind="ExternalOutput").ap()

    with tile.TileContext(nc) as tc, tc.tile_pool(name="sb", bufs=2) as sb, tc.tile_pool(
        name="ps", bufs=7, space="PSUM"
    ) as ps:
        ones_col = nc.const_aps.tensor(1.0, (128, 1), BF16)

        xs = {}

        def load_x(g):
            x = sb.tile([128, XW], BF16, tag="x", bufs=4, name=f"x{g}")
            hw = XW // 2
            nc.sync.dma_start(x[:, :hw], img[g, :, :hw])
            nc.sync.dma_start(x[:, hw:], img[g, :, hw:])
            xs[g] = x

        # DMA issue order: x0 (feeds the norm chain first), qpt, x1, x2;
        # later groups are issued inside the main loop with lookahead 3.
        load_x(0)
        qpt_sb = sb.tile([128, KC * 2 * I], BF16, tag="qpt", bufs=1, name="qpt_sb")
        for k in range(KC):
            nc.sync.dma_start(
                qpt_sb[:, k * 2 * I : (k + 1) * 2 * I], qpt[k * 128 : (k + 1) * 128, :]
            )
        load_x(1)
        load_x(2)

        # Persistent per-core accumulators: unnormalized dots + sumexp matrix.
        MSZ = [128, 128, TQ]
        outsb = [
            sb.tile([msz, NB], F32, tag=f"out{mi}", bufs=1, name=f"outsb{mi}")
            for mi, msz in enumerate(MSZ)
        ]
        semat = [
            sb.tile([msz, NB], F32, tag=f"se{mi}", bufs=1, name=f"semat{mi}")
            for mi, msz in enumerate(MSZ)
        ]

        # --- PE warm-up on a gpsimd-memset tile (no DMA dependency).
        wsrc = sb.tile([128, N], BF16, tag="warm", bufs=1, name="warmsrc")
        nc.gpsimd.memset(wsrc[:], 0.0)
        wps = ps.tile([1, N], F32, tag="wps", bufs=1, name="warmps")
        for i in range(N_WARM):
            nc.tensor.matmul(
                wps[:], ones_col, wsrc[:], start=(i == 0), stop=(i == N_WARM - 1)
            )

        def norm_chain(g, x):
            # Squares of the full [128, 4096] x tile split across engines,
            # 3-level pairwise bf16 add tree down to ssq [128, 512], then the
            # 128-partition reduction + rn = Exp(-0.5 * Ln(n2)) in bf16.
            sq = sb.tile([128, XW], BF16, tag="sq", bufs=2, name=f"sq{g}")
            h = XW // 4  # 1024
            nc.scalar.square(sq[:, 0 : 2 * h], x[:, 0 : 2 * h])
            nc.vector.tensor_mul(sq[:, 2 * h : 3 * h], x[:, 2 * h : 3 * h], x[:, 2 * h : 3 * h])
            nc.gpsimd.tensor_mul(sq[:, 3 * h : 4 * h], x[:, 3 * h : 4 * h], x[:, 3 * h : 4 * h])
            t2 = sb.tile([128, XW // 2], BF16, tag="t2", bufs=2, name=f"t2{g}")
            nc.vector.tensor_add(t2[:], sq[:, : XW // 2], sq[:, XW // 2 :])
            t1 = sb.tile([128, XW // 4], BF16, tag="t1", bufs=2, name=f"t1{g}")
            ssq = sb.tile([128, N], BF16, tag="ssq", bufs=2, name=f"ssq{g}")
            eng = nc.vector if g < N_MM_NORM else nc.gpsimd
            eng.tensor_add(t1[:], t2[:, : XW // 4], t2[:, XW // 4 :])
            eng.tensor_add(ssq[:], t1[:, :N], t1[:, N:])
            rn = sb.tile([128, N], BF16, tag="rn", bufs=3, name=f"rn{g}")
            if g < N_MM_NORM:
                # Ones-matmul partition reduce (PE is warming up anyway),
                # row Ln/Exp, then broadcast.
                n2p = ps.tile([1, N], F32, tag="wps", bufs=1, name=f"n2p{g}")
                nc.tensor.matmul(n2p[:], ones_col, ssq[:], start=True, stop=True)
                lnr = sb.tile([1, N], F32, tag="lnr", bufs=2, name=f"lnr{g}")
                nc.scalar.activation(lnr[:], n2p[:], LN)
                rnr = sb.tile([1, N], BF16, tag="rnr", bufs=2, name=f"rnr{g}")
                nc.scalar.activation(rnr[:], lnr[:], EXP, scale=-0.5)
                nc.gpsimd.partition_broadcast(rn[:], rnr[:], channels=128)
            else:
                # partition_all_reduce output is already broadcast across
                # partitions; Ln/Exp on the full tile costs the same as rows.
                n2 = sb.tile([128, N], F32, tag="n2", bufs=2, name=f"n2{g}")
                nc.gpsimd.partition_all_reduce(
                    n2[:], ssq[:], channels=128, reduce_op=bass_isa.ReduceOp.add
                )
                lnt = sb.tile([128, N], F32, tag="lnt", bufs=2, name=f"lnt{g}")
                nc.scalar.activation(lnt[:], n2[:], LN)
                nc.scalar.activation(rn[:], lnt[:], EXP, scale=-0.5)
            return rn

        def mm_chunk(g, x, coff, msz, nm):
            a = ps.tile([msz, N], F32, tag="sps", bufs=7, name=f"ps{nm}g{g}")
            for k in range(KC):
                nc.tensor.matmul(
                    a[:],
                    qpt_sb[:, k * 2 * I + coff : k * 2 * I + coff + msz],
                    x[:, k * N : (k + 1) * N],
                    start=(k == 0),
                    stop=(k == KC - 1),
                )
            return a

        def softmax_dot(g, mi, sqs, pap, msz):
            # sqs: scaled Q-side logits [msz, N] fp32; pap: P-side [msz, N]
            # (raw PSUM for m0/m1, rn-scaled SBUF for the tail); scale: rn
            # for m0/m1 (folded via F = E*rn), None for the tail.
            E = sb.tile([msz, N], BF16, tag="E", bufs=2, name=f"Eg{g}m{mi}")
            for h in range(2):
                nc.scalar.activation(
                    E[:, h * R : (h + 1) * R],
                    sqs[:, h * R : (h + 1) * R],
                    EXP,
                    accum_out=semat[mi][:msz, 2 * g + h : 2 * g + h + 1],
                )
            return E

        def dot_accum(g, mi, F, pap, msz):
            scr = sb.tile([msz, R], F32, tag="scr", bufs=2, name=f"scrg{g}m{mi}")
            for h in range(2):
                nc.vector.scalar_tensor_tensor(
                    out=scr[:],
                    in0=F[:, h * R : (h + 1) * R],
                    scalar=1.0,
                    in1=pap[:, h * R : (h + 1) * R],
                    op0=MULT,
                    op1=MULT,
                    accum_out=outsb[mi][:msz, 2 * g + h : 2 * g + h + 1],
                )

        def main_group(g, x, rn):
            # Tail chunk first: its partition-shift DMA then overlaps the two
            # full chunk-pairs' drains instead of sitting at the group's end.
            ta = mm_chunk(g, x, MCH_T, 2 * TQ, "t")
            ts = sb.tile([2 * TQ, N], F32, tag="tss", bufs=2, name=f"tsg{g}")
            nc.vector.tensor_mul(ts[:], ta[:], rn[: 2 * TQ, :])
            tp = sb.tile([TQ, N], F32, tag="tps", bufs=2, name=f"tpg{g}")
            nc.sync.dma_start(tp[:, :], ts[TQ : 2 * TQ, :])
            for mi in range(2):
                qa = mm_chunk(g, x, MCH_Q[mi], 128, f"q{mi}")
                pa = mm_chunk(g, x, MCH_P[mi], 128, f"p{mi}")
                sqs = sb.tile([128, N], F32, tag="sqs", bufs=2, name=f"sqsg{g}m{mi}")
                nc.vector.tensor_mul(sqs[:], qa[:], rn[:, :])
                E = softmax_dot(g, mi, sqs, pa, 128)
                F = sb.tile([128, N], BF16, tag="F", bufs=2, name=f"Fg{g}m{mi}")
                nc.vector.tensor_mul(F[:], E[:], rn[:, :])
                dot_accum(g, mi, F[:], pa[:], 128)
            Et = softmax_dot(g, 2, ts[:TQ, :], tp[:], TQ)
            dot_accum(g, 2, Et[:], tp[:], TQ)

        def finalize(p):
            # Softmax denominators for the 4 batches of pair p + store.
            offs = [0, 128, 256]
            for mi, msz in enumerate(MSZ):
                cs = slice(4 * p, 4 * p + 4)
                rec = sb.tile([msz, 4], F32, tag=f"rec{mi}", bufs=2, name=f"rec{mi}p{p}")
                nc.vector.reciprocal(rec[:], semat[mi][:msz, cs])
                fin = sb.tile([msz, 4], F32, tag=f"fin{mi}", bufs=2, name=f"fin{mi}p{p}")
                nc.vector.tensor_mul(fin[:], outsb[mi][:msz, cs], rec[:])
                nc.sync.dma_start(out[offs[mi] : offs[mi] + msz, cs], fin[:])

        for g in range(G):
            rn = norm_chain(g, xs[g])
            main_group(g, xs.pop(g), rn)
            if g % 2 == 1:
                finalize(g // 2)
            if g + 3 < G:
                load_x(g + 3)

    nc.compile()
    return nc


def _prepare(inputs):
    img = np.asarray(inputs["img"], np.float32)
    V = np.asarray(inputs["V"], np.float32)
    W1 = np.asarray(inputs["W1"], np.float32)
    W2 = np.asarray(inputs["W2"], np.float32)
    B, Cf, H, W = img.shape
    assert (B, Cf, H * W) == (N_CORES * NB, CF, R), img.shape

    import ml_dtypes

    vv = V.astype(np.float64)
    vv /= np.maximum(np.sqrt((vv * vv).sum(1, keepdims=True)), 1e-12)
    Q = vv @ W1.astype(np.float64)  # [I, CF]
    P = vv @ W2.astype(np.float64)
    # Column order: Q[0:128], Q[128:256], P[0:128], P[128:256], Q[256:], P[256:]
    stacked = np.concatenate(
        [Q[0:128], Q[128:256], P[0:128], P[128:256], Q[256:I], P[256:I]], axis=0
    )
    qpt = np.ascontiguousarray(stacked.T.astype(ml_dtypes.bfloat16))  # [CF, 624]

    # Per-core img: [G, 128, KC * 2 * R] bf16 so each group is one big tile
    # whose k-th 512-column slice is [128 f-rows, 2 batches x 256 r] and the
    # DRAM lines are 8 KB per partition row.
    imgb = img.reshape(B, Cf, H * W).astype(ml_dtypes.bfloat16)
    imgb = imgb.reshape(N_CORES, G, 2, KC, 128, R).transpose(0, 1, 4, 3, 2, 5)
    imgb = np.ascontiguousarray(imgb.reshape(N_CORES, G, 128, KC * 2 * R))
    in_maps = [{"img": imgb[c], "qpt": qpt} for c in range(N_CORES)]
    return in_maps


def run(inputs, **spmd_kwargs):
    """Run the kernel; returns (full_output [B, I], BassKernelResults)."""
    global _PROGRAM
    if _PROGRAM is None:
        _PROGRAM = _build_program()
    from concourse.bass_utils import run_bass_kernel_spmd

    in_maps = _prepare(inputs)
    res = run_bass_kernel_spmd(
        _PROGRAM, in_maps, core_ids=list(range(N_CORES)), **spmd_kwargs
    )
    out = np.concatenate(
        [np.asarray(res.results[c]["out"]).T for c in range(N_CORES)], axis=0
    )
    return np.ascontiguousarray(out, np.float32), res


def kernel(**inputs) -> np.ndarray:
    return run(inputs)[0]


# revision 9
# speedup vs baseline: 1.2836x; 1.2836x over previous
"""Trainium2 Bass kernel for nn_Classifier_custom_12936441496172.

Reference math (per batch b, with av = column-l2-normalized img_b [Cf, R]):
    A      = softmax_r( (vv @ W1) @ av )          # [I, R] attention over R
    F_p    = A @ av.T                             # [I, Cf]
    out[b] = rowsum( (vv @ W2) * F_p )            # [I]

Key identity: out[b, i] = sum_r A[i, r] * ((vv @ W2) @ av)[i, r], so F_p is
never materialized. (vv@W1)@av and (vv@W2)@av come from one stacked weight
matrix qpt, and the column normalization of av folds into a per-column scale
rn[r] = 1/||img_b[:, r]|| applied to the matmul outputs.

Sharding: data-parallel over batch across 8 NeuronCores (16 batches each),
parameters replicated. Parameter prep (vv @ W1/W2, < 1% of FLOPs) on host.

Device kernel per core: 8 groups of 2 batches (N = 512 matmul free dim).
  - img arrives as one [128, 4096] bf16 tile per group (two dma_starts with
    4 KB per-partition lines, issued with a rolling 3-group lookahead so
    drain-critical DMAs never queue behind them).
  - norms: squares at [128, 1024-2048] grain split across ACT/DVE/GpSimd,
    a pairwise bf16 add tree, then the 128-partition reduction: an
    accumulating ones-vector matmul + row Ln/Exp + partition_broadcast for
    the first two (latency-critical) groups, gpsimd partition_all_reduce
    (output already broadcast) + full-tile Ln/Exp for the rest.
    rn = Exp(-0.5 * Ln(n2)) is produced in bf16.
  - main: 5 m-chunks of the 624 stacked rows (tail packs Q/P 56-row
    remainders), each 8 accumulating bf16 matmuls.
  - drains per chunk-pair: sqs = qa*rn (DVE, reads PSUM), E = Exp(sqs) in
    bf16 with fp32 free-axis accum -> sumexp columns (ACT), F = E*rn (DVE
    bf16 2x), then scalar_tensor_tensor F*pa (pa straight from PSUM) with
    accum -> unnormalized output column.
  - softmax denominators are applied per PAIR of groups (reciprocal +
    multiply on [*, 4] tiles + output DMA), so the kernel tail is only the
    last group's drain chain.
All ACT functions (Square/Ln/Exp) are pinned to the one act-func set that
contains all three, so ACT performs a single table load. PE warm-up: ~14
dummy matmuls on a gpsimd-memset tile while the first DMAs are in flight.
Logits are ~N(0,1) so softmax max-subtraction is skipped; exp cannot
overflow fp32.
"""

import numpy as np

_PROGRAM = None

# Problem geometry (hardcoded per contract; kernel.py must be self-contained)
N_CORES = 8
NB = 16          # batches per core
R = 256          # H * W
CF = 1024        # feature channels
KC = CF // 128   # 8 contraction chunks
I = 312          # attributes
G = NB // 2      # groups of 2 batches
N = 2 * R        # matmul moving free dim (2 batches)
TQ = I - 256     # 56-row tails
XW = KC * N      # x tile width (4096)
# m-chunk column offsets in the host-reordered qpt
MCH_Q = [0, 128]       # Q rows 0:128, 128:256
MCH_P = [256, 384]     # P rows 0:128, 128:256
MCH_T = 512            # Q rows 256:312 at cols 512:568, P rows at 568:624
N_WARM = 14
N_MM_NORM = 2    # groups whose norm reduction rides the TensorEngine


def _pin_act_tables(arch):
    """Blank every act-func set except natural_log_exp_and_others (which
    contains Square, Ln and Exp) in the cached table dict, so the table-load
    pass assigns all our activations to that one set and the ACT engine
    performs a single table load instead of flipping per Ln/Exp pair.
    Indices (insertion order) are preserved, so the emitted set id still
    refers to the same canonical act_info.json entry."""
    from concourse.hw_specs import get_activation_tables

    tabs = get_activation_tables(arch)
    target = "natural_log_exp_and_others"
    if target in tabs:
        for name in tabs:
            if name != target:
                tabs[name] = set()


def _build_program():
    import concourse.tile as tile
    from concourse import bacc, bass_isa, mybir

    F32 = mybir.dt.float32
    BF16 = mybir.dt.bfloat16
    MULT = mybir.AluOpType.mult
    EXP = mybir.ActivationFunctionType.Exp
    LN = mybir.ActivationFunctionType.Ln

    nc = bacc.Bacc(
        "TRN2",
        target_bir_lowering=False,
        debug=False,
        enable_asserts=False,
        num_devices=N_CORES,
    )
    _pin_act_tables(nc.m.arch)

    img = nc.dram_tensor("img", [G, 128, XW], BF16, kind="ExternalInput").ap()
    qpt = nc.dram_tensor("qpt", [CF, 2 * I], BF16, kind="ExternalInput").ap()
    out = nc.dram_tensor("out", [I, NB], F32, kind="ExternalOutput").ap()

    with tile.TileContext(nc) as tc, tc.tile_pool(name="sb", bufs=2) as sb, tc.tile_pool(
        name="ps", bufs=7, space="PSUM"
    ) as ps:
        ones_col = nc.const_aps.tensor(1.0, (128, 1), BF16)

        xs = {}

        def load_x(g):
            x = sb.tile([128, XW], BF16, tag="x", bufs=5, name=f"x{g}")
            hw = XW // 2
            nc.sync.dma_start(x[:, :hw], img[g, :, :hw])
            nc.sync.dma_start(x[:, hw:], img[g, :, hw:])
            xs[g] = x

        # DMA issue order: x0 (feeds the norm chain first), qpt, x1..x3;
        # later groups are issued inside the main loop with lookahead 4.
        load_x(0)
        qpt_sb = sb.tile([128, KC * 2 * I], BF16, tag="qpt", bufs=1, name="qpt_sb")
        for k in range(KC):
            nc.sync.dma_start(
                qpt_sb[:, k * 2 * I : (k + 1) * 2 * I], qpt[k * 128 : (k + 1) * 128, :]
            )
        load_x(1)
        load_x(2)
        load_x(3)

        # Persistent per-core accumulators: unnormalized dots + sumexp matrix.
        MSZ = [128, 128, TQ]
        outsb = [
            sb.tile([msz, NB], F32, tag=f"out{mi}", bufs=1, name=f"outsb{mi}")
            for mi, msz in enumerate(MSZ)
        ]
        semat = [
            sb.tile([msz, NB], F32, tag=f"se{mi}", bufs=1, name=f"semat{mi}")
            for mi, msz in enumerate(MSZ)
        ]

        # --- PE warm-up on a gpsimd-memset tile (no DMA dependency).
        wsrc = sb.tile([128, N], BF16, tag="warm", bufs=1, name="warmsrc")
        nc.gpsimd.memset(wsrc[:], 0.0)
        wps = ps.tile([1, N], F32, tag="wps", bufs=1, name="warmps")
        for i in range(N_WARM):
            nc.tensor.matmul(
                wps[:], ones_col, wsrc[:], start=(i == 0), stop=(i == N_WARM - 1)
            )

        def norm_chain(g, x):
            # Squares of the full [128, 4096] x tile split across engines,
            # 3-level pairwise bf16 add tree down to ssq [128, 512], then a
            # single ones-matmul partition reduce, row Ln/Exp (rn in bf16),
            # and a gpsimd partition broadcast.
            sq = sb.tile([128, XW], BF16, tag="sq", bufs=2, name=f"sq{g}")
            h = XW // 4  # 1024
            nc.scalar.square(sq[:, 0 : 2 * h], x[:, 0 : 2 * h])
            nc.vector.tensor_mul(sq[:, 2 * h : 3 * h], x[:, 2 * h : 3 * h], x[:, 2 * h : 3 * h])
            nc.gpsimd.tensor_mul(sq[:, 3 * h : 4 * h], x[:, 3 * h : 4 * h], x[:, 3 * h : 4 * h])
            t2 = sb.tile([128, XW // 2], BF16, tag="t2", bufs=2, name=f"t2{g}")
            nc.vector.tensor_add(t2[:], sq[:, : XW // 2], sq[:, XW // 2 :])
            t1 = sb.tile([128, XW // 4], BF16, tag="t1", bufs=2, name=f"t1{g}")
            ssq = sb.tile([128, N], BF16, tag="ssq", bufs=2, name=f"ssq{g}")
            nc.gpsimd.tensor_add(t1[:], t2[:, : XW // 4], t2[:, XW // 4 :])
            nc.gpsimd.tensor_add(ssq[:], t1[:, :N], t1[:, N:])
            rn = sb.tile([128, N], BF16, tag="rn", bufs=4, name=f"rn{g}")
            n2p = ps.tile([1, N], F32, tag="wps", bufs=1, name=f"n2p{g}")
            nc.tensor.matmul(n2p[:], ones_col, ssq[:], start=True, stop=True)
            lnr = sb.tile([1, N], F32, tag="lnr", bufs=2, name=f"lnr{g}")
            nc.scalar.activation(lnr[:], n2p[:], LN)
            rnr = sb.tile([1, N], BF16, tag="rnr", bufs=2, name=f"rnr{g}")
            nc.scalar.activation(rnr[:], lnr[:], EXP, scale=-0.5)
            nc.gpsimd.partition_broadcast(rn[:], rnr[:], channels=128)
            return rn

        def mm_chunk(g, x, coff, msz, nm):
            a = ps.tile([msz, N], F32, tag="sps", bufs=7, name=f"ps{nm}g{g}")
            for k in range(KC):
                nc.tensor.matmul(
                    a[:],
                    qpt_sb[:, k * 2 * I + coff : k * 2 * I + coff + msz],
                    x[:, k * N : (k + 1) * N],
                    start=(k == 0),
                    stop=(k == KC - 1),
                )
            return a

        def softmax_dot(g, mi, sqs, pap, msz):
            # sqs: scaled Q-side logits [msz, N] fp32; pap: P-side [msz, N]
            # (raw PSUM for m0/m1, rn-scaled SBUF for the tail); scale: rn
            # for m0/m1 (folded via F = E*rn), None for the tail.
            E = sb.tile([msz, N], BF16, tag="E", bufs=2, name=f"Eg{g}m{mi}")
            for h in range(2):
                nc.scalar.activation(
                    E[:, h * R : (h + 1) * R],
                    sqs[:, h * R : (h + 1) * R],
                    EXP,
                    accum_out=semat[mi][:msz, 2 * g + h : 2 * g + h + 1],
                )
            return E

        def dot_accum(g, mi, F, pap, msz):
            scr = sb.tile([msz, R], F32, tag="scr", bufs=2, name=f"scrg{g}m{mi}")
            for h in range(2):
                nc.vector.scalar_tensor_tensor(
                    out=scr[:],
                    in0=F[:, h * R : (h + 1) * R],
                    scalar=1.0,
                    in1=pap[:, h * R : (h + 1) * R],
                    op0=MULT,
                    op1=MULT,
                    accum_out=outsb[mi][:msz, 2 * g + h : 2 * g + h + 1],
                )

        def main_group(g, x, rn):
            # Tail chunk first: its partition-shift DMA then overlaps the two
            # full chunk-pairs' drains instead of sitting at the group's end.
            ta = mm_chunk(g, x, MCH_T, 2 * TQ, "t")
            ts = sb.tile([2 * TQ, N], F32, tag="tss", bufs=2, name=f"tsg{g}")
            nc.vector.tensor_mul(ts[:], ta[:], rn[: 2 * TQ, :])
            tp = sb.tile([TQ, N], F32, tag="tps", bufs=2, name=f"tpg{g}")
            nc.sync.dma_start(tp[:, :], ts[TQ : 2 * TQ, :])
            for mi in range(2):
                qa = mm_chunk(g, x, MCH_Q[mi], 128, f"q{mi}")
                pa = mm_chunk(g, x, MCH_P[mi], 128, f"p{mi}")
                sqs = sb.tile([128, N], F32, tag="sqs", bufs=2, name=f"sqsg{g}m{mi}")
                nc.vector.tensor_mul(sqs[:], qa[:], rn[:, :])
                E = softmax_dot(g, mi, sqs, pa, 128)
                F = sb.tile([128, N], BF16, tag="F", bufs=2, name=f"Fg{g}m{mi}")
                nc.vector.tensor_mul(F[:], E[:], rn[:, :])
                dot_accum(g, mi, F[:], pa[:], 128)
            Et = softmax_dot(g, 2, ts[:TQ, :], tp[:], TQ)
            dot_accum(g, 2, Et[:], tp[:], TQ)

        def finalize(p):
            # Softmax denominators for the 4 batches of pair p + store.
            offs = [0, 128, 256]
            for mi, msz in enumerate(MSZ):
                cs = slice(4 * p, 4 * p + 4)
                rec = sb.tile([msz, 4], F32, tag=f"rec{mi}", bufs=2, name=f"rec{mi}p{p}")
                nc.vector.reciprocal(rec[:], semat[mi][:msz, cs])
                fin = sb.tile([msz, 4], F32, tag=f"fin{mi}", bufs=2, name=f"fin{mi}p{p}")
                nc.vector.tensor_mul(fin[:], outsb[mi][:msz, cs], rec[:])
                nc.sync.dma_start(out[offs[mi] : offs[mi] + msz, cs], fin[:])

        # Software pipeline: norm chain for group g+1 is emitted right after
        # group g's matmuls, so its ones-matmul lands between the two MM
        # streams on the PE queue with the add tree already complete, and
        # rn(g+1) is ready ~1 us later — before group g+1's first drain.
        rns = {0: norm_chain(0, xs[0])}
        for g in range(G):
            main_group(g, xs.pop(g), rns.pop(g))
            if g + 1 < G:
                rns[g + 1] = norm_chain(g + 1, xs[g + 1])
            if g % 2 == 1:
                finalize(g // 2)
            if g + 4 < G:
                load_x(g + 4)

    nc.compile()
    return nc


def _prepare(inputs):
    img = np.asarray(inputs["img"], np.float32)
    V = np.asarray(inputs["V"], np.float32)
    W1 = np.asarray(inputs["W1"], np.float32)
    W2 = np.asarray(inputs["W2"], np.float32)
    B, Cf, H, W = img.shape
    assert (B, Cf, H * W) == (N_CORES * NB, CF, R), img.shape

    import ml_dtypes

    vv = V.astype(np.float64)
    vv /= np.maximum(np.sqrt((vv * vv).sum(1, keepdims=True)), 1e-12)
    Q = vv @ W1.astype(np.float64)  # [I, CF]
    P = vv @ W2.astype(np.float64)
    # Column order: Q[0:128], Q[128:256], P[0:128], P[128:256], Q[256:], P[256:]
    stacked = np.concatenate(
        [Q[0:128], Q[128:256], P[0:128], P[128:256], Q[256:I], P[256:I]], axis=0
    )
    qpt = np.ascontiguousarray(stacked.T.astype(ml_dtypes.bfloat16))  # [CF, 624]

    # Per-core img: [G, 128, KC * 2 * R] bf16 so each group is one big tile
    # whose k-th 512-column slice is [128 f-rows, 2 batches x 256 r] and the
    # DRAM lines are 8 KB per partition row.
    imgb = img.reshape(B, Cf, H * W).astype(ml_dtypes.bfloat16)
    imgb = imgb.reshape(N_CORES, G, 2, KC, 128, R).transpose(0, 1, 4, 3, 2, 5)
    imgb = np.ascontiguousarray(imgb.reshape(N_CORES, G, 128, KC * 2 * R))
    in_maps = [{"img": imgb[c], "qpt": qpt} for c in range(N_CORES)]
    return in_maps


def run(inputs, **spmd_kwargs):
    """Run the kernel; returns (full_output [B, I], BassKernelResults)."""
    global _PROGRAM
    if _PROGRAM is None:
        _PROGRAM = _build_program()
    from concourse.bass_utils import run_bass_kernel_spmd

    in_maps = _prepare(inputs)
    res = run_bass_kernel_spmd(
        _PROGRAM, in_maps, core_ids=list(range(N_CORES)), **spmd_kwargs
    )
    out = np.concatenate(
        [np.asarray(res.results[c]["out"]).T for c in range(N_CORES)], axis=0
    )
    return np.ascontiguousarray(out, np.float32), res


def kernel(**inputs) -> np.ndarray:
    return run(inputs)[0]
